# revision 32
# baseline (speedup 1.0000x reference)
"""Trainium2 Bass kernel for nn_BilateralCorrelationFlex (8-core SPMD)."""

import os
import numpy as np
import concourse.bacc as bacc
import concourse.bass as bass
import concourse.mybir as mybir
import concourse.tile as tile
from concourse import library_config
from concourse.masks import make_identity
from concourse.bass_test_utils import get_trn_type

F32 = mybir.dt.float32
F32R = mybir.dt.float32r
I16 = mybir.dt.int16

B, C, PREV, N, M1, M2 = 2, 16, 16, 16384, 4096, 4096
D1, CORR, FILT = 4, 15, 15
NEG = 0.1
NSLOT = 4608           # 36 windows of 128 slots (4097 real)
NWIN_L = 9             # windows per core (sharded splat)
NSUB_L = 36            # subwindows per core
NROW_L = NWIN_L * 128  # 1152 local slot rows
SLICES = 4
NSL = M1 // SLICES     # 1024 output columns per core
NBLK = NSL // 128      # 8
KOCT = [list(range(0, 8)), list(range(8, 15))]
LT = int(os.environ.get("KLT", "8"))    # splat q-tiles per dma_gather op
GCH = int(os.environ.get("KGCH", "8"))  # g1/g2 k-items per dma_gather op
KQN = int(os.environ.get("KQN", "2"))   # SWDGE queues (round-robin gathers)
KSP = bool(int(os.environ.get("KSP", "1")))  # single_packet flag for gathers

AluOp = mybir.AluOpType
ALL_GROUPS = [[0, 1, 2, 3], [4, 5, 6, 7]]


# ---------------------------------------------------------------- host prep
def _dev_idx_layout(flat):
    """flat want-list (num,) -> device idx tile (128, num//16) int16.
    Item i is read from [i % 16, i // 16] of the 16-row block, tiled 8x."""
    num = flat.shape[0]
    assert num % 128 == 0
    blk = np.zeros((16, num // 16), np.int16)
    blk[np.arange(num) % 16, np.arange(num) // 16] = flat
    return np.tile(blk, (8, 1))


def host_prep(inputs):
    f1 = np.asarray(inputs["feat1"], np.float32)
    f2 = np.asarray(inputs["feat2"], np.float32)
    prev = np.asarray(inputs["prev_corr_feat"], np.float32)
    bary = np.asarray(inputs["barycentric1"], np.float32)
    lat = np.asarray(inputs["lattice_offset1"])
    pc1 = np.asarray(inputs["pc1_corr_indices"])
    pc2 = np.asarray(inputs["pc2_corr_indices"])
    cw0 = np.asarray(inputs["cw0"], np.float32); cb0 = np.asarray(inputs["cb0"], np.float32)
    cw1 = np.asarray(inputs["cw1"], np.float32); cb1 = np.asarray(inputs["cb1"], np.float32)
    bw0 = np.asarray(inputs["bw0"], np.float32); bb0 = np.asarray(inputs["bb0"], np.float32)
    bw1 = np.asarray(inputs["bw1"], np.float32); bb1 = np.asarray(inputs["bb1"], np.float32)

    # --- per (batch, slice) sorted subwindow contribution lists (index math) ---
    lists = {}
    for b in range(B):
        idxl = (lat[b].reshape(-1) - b * (M1 + 1) + 1).astype(np.int64)  # slot, 1..4096
        order = np.argsort(idxl, kind="stable")
        ss = idxl[order]
        for s in range(SLICES):
            for lsw in range(NSUB_L):
                gsw = NSUB_L * s + lsw
                lo, hi = gsw * 32, gsw * 32 + 32
                a = np.searchsorted(ss, lo, "left")
                z = np.searchsorted(ss, hi, "left")
                lists[(b, s, lsw)] = (order[a:z], ss[a:z] - lo)

    ntile = [max(1, max((len(lists[(b, s, lsw)][0]) + 127) // 128
                        for b in range(B) for s in range(SLICES)))
             for lsw in range(NSUB_L)]
    TTS = sum(ntile)
    if TTS % LT:
        pad = LT - TTS % LT
        ntile[NSUB_L - 1] += pad
        TTS += pad
    lsub = np.concatenate([np.full(ntile[lsw], lsw, np.int32)
                           for lsw in range(NSUB_L)])
    # per-tile start/stop flags for its psum quarter (32-slot subwindow)
    first = np.zeros(TTS, bool); last = np.zeros(TTS, bool)
    pos = 0
    for lsw in range(NSUB_L):
        first[pos] = True; last[pos + ntile[lsw] - 1] = True
        pos += ntile[lsw]

    shared = dict(TTS=TTS, lsub=lsub, first=first, last=last)

    # --- weights (device layout) ---
    cw0A = np.zeros((128, 128), np.float32)     # [chunk*32 cols] rows j=32k+c in chunk
    for k in range(CORR):
        for c in range(32):
            j = 32 * k + c
            cw0A[j % 128, 32 * (j // 128):32 * (j // 128) + 32] = cw0[:, c, k]
    cw0B = np.zeros((128, 64), np.float32)      # oct o cols 32o:32o+32, rows 16ki+c
    for o in range(2):
        for ki, k in enumerate(KOCT[o]):
            for c in range(16):
                cw0B[16 * ki + c, 32 * o:32 * o + 32] = cw0[:, 32 + c, k]
    bw0T = np.zeros((32, CORR * 64), np.float32)
    for f in range(FILT):
        bw0T[:, 64 * f:64 * f + 64] = bw0[:, :, f].T
    consts = dict(
        cw0A=cw0A, cw0B=cw0B, cw1T=np.ascontiguousarray(cw1.T), bw0T=bw0T,
        bw1T=np.ascontiguousarray(bw1.T),
        cb0c=cb0.reshape(32, 1), cb1c=cb1.reshape(32, 1),
        bb0c=bb0.reshape(64, 1), bb1c=bb1.reshape(64, 1),
        iota32=np.broadcast_to(np.arange(32, dtype=np.float32), (128, 32)).copy())

    percore = []
    for core in range(8):
        b, s = core // SLICES, core % SLICES
        # tmat: row n = [prev[:, n] (16), bary[0..3, n] (4), 0 pad] stride 64
        tmat = np.zeros((N, 64), np.float32)
        tmat[:, 0:16] = prev[b].T
        tmat[:, 16:20] = bary[b].T
        s2tab = np.zeros((4224, 64), np.float32)
        s2tab[1:M2 + 1, 0:16] = f2[b].T
        f1glob = np.zeros((NSLOT, 16), np.float32)
        f1glob[1:M1 + 1] = f1[b].T
        f1loc = np.ascontiguousarray(f1glob[NROW_L * s:NROW_L * (s + 1)])

        gq = np.zeros((TTS, 128), np.int64)
        wseg = np.full((TTS, 128), 99.0, np.float32)
        jsel = np.zeros((TTS, 128, 4), np.float32)
        pos = 0
        for lsw in range(NSUB_L):
            qs, segs = lists[(b, s, lsw)]
            for t in range(ntile[lsw]):
                q_t = qs[t * 128:(t + 1) * 128]
                s_t = segs[t * 128:(t + 1) * 128]
                m = len(q_t)
                gq[pos, :m] = q_t % N
                wseg[pos, :m] = s_t
                jsel[pos, np.arange(m), q_t // N] = 1.0
                pos += 1
        assert pos == TTS
        # device layouts
        gq_dev = np.zeros((128, TTS * 8), np.int16)
        for g in range(TTS // LT):
            flat = gq[g * LT:(g + 1) * LT].reshape(-1)  # item i = t*128+p
            gq_dev[:, g * LT * 8:(g + 1) * LT * 8] = _dev_idx_layout(flat)
        wseg_dev = np.ascontiguousarray(wseg.T)                     # (128, TTS)
        jsel_dev = np.ascontiguousarray(
            jsel.transpose(1, 0, 2).reshape(128, TTS * 4))          # (128, TTS*4)

        n0 = s * NSL
        g2i = (pc2[b, :, :, n0:n0 + NSL] + 1).astype(np.int64)      # (F, K, NSL)
        g2i = g2i.reshape(CORR * FILT, NBLK, 128)
        g2_dev = np.zeros((128, NBLK * 1800), np.int16)
        for blk in range(NBLK):
            col = blk * 1800
            for a in range(0, 225, GCH):
                nk = min(GCH, 225 - a)
                flat = g2i[a:a + nk, blk, :].reshape(-1)            # i = fk_loc*128+n
                g2_dev[:, col:col + nk * 8] = _dev_idx_layout(flat)
                col += nk * 8
            assert col == (blk + 1) * 1800
        g1i = (pc1[b, :, n0:n0 + NSL] + 1).astype(np.int64).reshape(CORR, NBLK, 128)
        g1_dev = np.zeros((128, NBLK * 120), np.int16)
        for blk in range(NBLK):
            col = blk * 120
            for a in range(0, 15, GCH):
                nk = min(GCH, 15 - a)
                flat = g1i[a:a + nk, blk, :].reshape(-1)
                g1_dev[:, col:col + nk * 8] = _dev_idx_layout(flat)
                col += nk * 8

        percore.append(dict(tmat=tmat, s2tab=s2tab, f1loc=f1loc,
                            gq16=gq_dev, wseg=wseg_dev, jsel=jsel_dev,
                            g2i16=g2_dev, g1i16=g1_dev, **consts))
    return shared, percore


# ------------------------------------------------------- patched dma_gather
def emit_dma_gather(gp, out_ap, in_ap, idxs_ap, num_idxs, elem_size, elem_step, queue_num=0, single_packet=None):
    """bass.BassGpSimd.dma_gather minus the elem_size%256 assert
    (stride stays 256B-aligned, which is the real ISA constraint)."""
    from concourse import ap_utils
    from concourse.bass import exact_div
    if single_packet is None:
        single_packet = KSP
    assert idxs_ap.dtype == I16
    assert in_ap.space == bass.MemorySpace.DRAM
    assert idxs_ap.space == bass.MemorySpace.SBUF
    assert out_ap.space == bass.MemorySpace.SBUF
    assert ap_utils.ap_is_contiguous(in_ap.ap[1:])
    assert ap_utils.ap_is_contiguous(out_ap.ap[1:])
    assert ap_utils.ap_is_contiguous(idxs_ap.ap[1:])
    assert num_idxs % 128 == 0
    assert in_ap.ap[-1][1] == out_ap.ap[-1][1] == elem_size
    assert out_ap.ap[0][1] * out_ap.ap[1][1] == num_idxs
    assert in_ap.ap[0][0] == elem_step
    stride_bytes_256 = exact_div(elem_step * mybir.dt.size(in_ap.dtype), 256)
    _in_ap = gp.lower_ap_dma(in_ap, for_custom_bir_dma=True)
    inst = gp.add_instruction(
        mybir.InstDMAGatherAnt(
            name=gp.bass.get_next_instruction_name(),
            ins=[*_in_ap, gp.lower_ap(idxs_ap),
                 gp.lower_val_access(gp.to_reg(num_idxs))],
            outs=[gp.lower_ap(out_ap)],
            transpose=False, num_idxs=num_idxs, elem_size=elem_size,
            stride_bytes_256=stride_bytes_256, gen_mode=0,
            single_packet=single_packet, queue_num=queue_num,
            sbuf_tokens_per_rank=0, sbuf_free_dim_per_rank=0,
            sbuf_free_dim_pad_per_rank=0, sbuf_byte_offset=0))
    return inst


# ---------------------------------------------------------------- builder
def build_nc(TTS, lsub, first, last, mm_dt=F32, reps=1, only=None, coll=True):
    do_gather = only in (None, "gather")
    do_compute = only in (None, "compute")
    nc = bacc.Bacc(get_trn_type() or "TRN2", target_bir_lowering=False,
                   debug=False, enable_asserts=True, num_devices=8,
                   num_swdge_queues=KQN)
    qrr = [0]

    def next_q():
        q = qrr[0]
        qrr[0] = (q + 1) % KQN
        return q
    dt_in = {}

    def din(name, shape, dtype=F32):
        dt_in[name] = nc.dram_tensor(name, list(shape), dtype, kind="ExternalInput").ap()
        return dt_in[name]

    tmat = din("tmat", (N, 64)); s2tab = din("s2tab", (4224, 64))
    f1loc = din("f1loc", (NROW_L, 16))
    gq16 = din("gq16", (128, TTS * 8), I16)
    wseg = din("wseg", (128, TTS)); jsel = din("jsel", (128, TTS * 4))
    g2i16 = din("g2i16", (128, NBLK * 1800), I16)
    g1i16 = din("g1i16", (128, NBLK * 120), I16)
    cw0A = din("cw0A", (128, 128)); cw0B = din("cw0B", (128, 64))
    cw1T = din("cw1T", (32, 32)); bw0T = din("bw0T", (32, CORR * 64))
    bw1T = din("bw1T", (64, 64))
    cb0c = din("cb0c", (32, 1)); cb1c = din("cb1c", (32, 1))
    bb0c = din("bb0c", (64, 1)); bb1c = din("bb1c", (64, 1))
    iota32 = din("iota32", (128, 32))

    y_out = nc.dram_tensor("y_out", [64, NSL], F32, kind="ExternalOutput").ap()
    spl_loc = nc.dram_tensor("spl_loc", [NROW_L, 64], F32).ap()
    spl_full = nc.dram_tensor("spl_full", [NSLOT, 64], F32).ap()

    ngrp = TTS // LT
    V = nc.vector

    with tile.TileContext(nc) as tc:
        with (tc.tile_pool(name="cst", bufs=1) as cst,
              tc.tile_pool(name="sidx", bufs=2) as sidx,
              tc.tile_pool(name="sg2", bufs=4) as sg2,
              tc.tile_pool(name="sgw", bufs=2) as sgw,
              tc.tile_pool(name="swk", bufs=2) as swk,
              tc.tile_pool(name="srh", bufs=2) as srh,
              tc.tile_pool(name="psw_p", bufs=1, space="PSUM") as psw_p,
              tc.tile_pool(name="ptr_p", bufs=2, space="PSUM") as ptr_p,
              tc.tile_pool(name="pa_p", bufs=1, space="PSUM") as pa_p,
              tc.tile_pool(name="px_p", bufs=1, space="PSUM") as px_p,
              tc.tile_pool(name="px2_p", bufs=1, space="PSUM") as px2_p,
              tc.tile_pool(name="py_p", bufs=2, space="PSUM") as py_p):
            nc.gpsimd.load_library(library_config.mlp)

            # ---------------- constants to SBUF ----------------
            def load_const(ap_in, shape, nm, dtype=F32):
                t = cst.tile(list(shape), dtype, tag=nm, name=nm)
                nc.sync.dma_start(out=t[:], in_=ap_in[:])
                return t

            gq_sb = load_const(gq16, (128, TTS * 8), "gq_sb", I16)
            ws_sb = load_const(wseg, (128, TTS), "ws_sb")
            js_sb = load_const(jsel, (128, TTS * 4), "js_sb")
            g1i_sb = load_const(g1i16, (128, NBLK * 120), "g1i_sb", I16)
            io_sb = load_const(iota32, (128, 32), "io_sb")
            cb0_sb = load_const(cb0c, (32, 1), "cb0_sb")
            cb1_sb = load_const(cb1c, (32, 1), "cb1_sb")
            bb0_sb = load_const(bb0c, (64, 1), "bb0_sb")
            bb1_sb = load_const(bb1c, (64, 1), "bb1_sb")
            ident = cst.tile([128, 128], F32)
            make_identity(nc, ident[:])

            def to_r(name, src_ap, shape):
                f = cst.tile(list(shape), F32, tag=name + "_f")
                nc.sync.dma_start(out=f[:], in_=src_ap[:])
                r = cst.tile(list(shape), mm_dt, tag=name + "_r")
                V.tensor_copy(out=r[:], in_=f[:])
                return r

            cw0A_r = to_r("cw0A", cw0A, (128, 128))
            cw0B_r = to_r("cw0B", cw0B, (128, 64))
            cw1_r = to_r("cw1T", cw1T, (32, 32))
            bw0_r = to_r("bw0T", bw0T, (32, CORR * 64))
            bw1_r = to_r("bw1T", bw1T, (64, 64))
            id32_r = cst.tile([32, 32], mm_dt)
            V.tensor_copy(out=id32_r[:], in_=ident[0:32, 0:32])

            g2tiles = {}

            KES = int(os.environ.get("KES", "16"))  # gather-only elem probe

            def g2_gather(blk):
                es = 16 if do_compute else KES
                gt = sg2.tile([128, 225, es], F32, tag="g2all")
                if not do_gather:
                    V.memset(gt[:, 0:1, :], 0.0)
                if do_gather:
                    it = sidx.tile([128, 1800], I16, tag="g2idx")
                    nc.sync.dma_start(out=it[:], in_=g2i16[:, blk * 1800:(blk + 1) * 1800])
                    col = 0
                    for a in range(0, 225, GCH):
                        nk = min(GCH, 225 - a)
                        emit_dma_gather(nc.gpsimd, gt[:, a:a + nk, :], s2tab[:, 0:es],
                                        it[:, col:col + nk * 8],
                                        num_idxs=nk * 128, elem_size=es, elem_step=64,
                                        queue_num=next_q())
                        col += nk * 8
                g2tiles[blk] = gt

            for rep_i in range(reps):
                # ---------------- splat phase ----------------
                pos = 0
                psw = {}
                for w in range(NWIN_L):
                    psw[w] = psw_p.tile([128, 17], F32, tag="psw", name=f"psw{w}_{rep_i}")
                emitted_norm = set()
                for g in range(ngrp):
                    gt = sgw.tile([128, LT, 20], F32, tag="tg")
                    if do_gather:
                        emit_dma_gather(nc.gpsimd, gt[:], tmat[:, 0:20],
                                        gq_sb[:, g * LT * 8:(g + 1) * LT * 8],
                                        num_idxs=LT * 128, elem_size=20, elem_step=64,
                                        queue_num=next_q())
                    else:
                        V.memset(gt[:, 0:1, :], 0.0)
                    if not do_compute:
                        continue
                    # bsel = sum_j G[:, :, 16+j] * jsel
                    tmpj = swk.tile([128, LT, 4], F32, tag="tmpj")
                    js_view = js_sb[:, g * LT * 4:(g + 1) * LT * 4]
                    V.tensor_tensor(out=tmpj[:], in0=gt[:, :, 16:20],
                                    in1=js_view.rearrange("p (t j) -> p t j", j=4),
                                    op=AluOp.mult)
                    bsel = swk.tile([128, LT, 1], F32, tag="bsel")
                    V.reduce_sum(out=bsel[:], in_=tmpj[:], axis=mybir.AxisListType.X)
                    rhs = srh.tile([128, LT, 17], F32, tag="rhs")
                    V.tensor_tensor(out=rhs[:, :, 0:16], in0=gt[:, :, 0:16],
                                    in1=bsel[:].broadcast_to([128, LT, 16]),
                                    op=AluOp.mult)
                    V.tensor_copy(out=rhs[:, :, 16:17], in_=bsel[:])
                    Z = srh.tile([128, LT, 32], F32, tag="Z")
                    V.tensor_tensor(
                        out=Z[:],
                        in0=ws_sb[:, g * LT:(g + 1) * LT].unsqueeze(-1).broadcast_to([128, LT, 32]),
                        in1=io_sb[:].unsqueeze(1).broadcast_to([128, LT, 32]),
                        op=AluOp.is_equal)
                    for t in range(LT):
                        ti = pos + t
                        lsw = int(lsub[ti]); w, q = lsw // 4, lsw % 4
                        nc.tensor.matmul(out=psw[w][32 * q:32 * q + 32, :],
                                         lhsT=Z[:, t, :], rhs=rhs[:, t, :],
                                         start=bool(first[ti]), stop=bool(last[ti]),
                                         tile_position=(0, 32 * q))
                    pos += LT
                    # emit window normalize when its last subwindow's last tile emitted
                    for w in range(NWIN_L):
                        if w in emitted_norm:
                            continue
                        wend = int(np.max(np.where(lsub // 4 == w)[0]))
                        if wend < pos:
                            emitted_norm.add(w)
                            madd = swk.tile([128, 1], F32, tag="madd")
                            V.tensor_scalar(out=madd[:], in0=psw[w][:, 16:17],
                                            scalar1=1e-5, scalar2=None, op0=AluOp.add)
                            rec = swk.tile([128, 1], F32, tag="rec")
                            V.reciprocal(out=rec[:], in_=madd[:])
                            stage = srh.tile([128, 64], F32, tag="stage")
                            V.memset(stage[:, 32:64], 0.0)
                            V.tensor_scalar(out=stage[:, 0:16], in0=psw[w][:, 0:16],
                                            scalar1=rec[:], scalar2=None, op0=AluOp.mult)
                            nc.sync.dma_start(out=stage[:, 16:32],
                                              in_=f1loc[128 * w:128 * (w + 1), :])
                            nc.sync.dma_start(out=spl_loc[128 * w:128 * (w + 1), :],
                                              in_=stage[:])
                # prefetch g2 blocks 0,1 while splat compute / collective run
                g2_gather(0)
                g2_gather(1)
                # share splats across the batch group
                if coll:
                    nc.gpsimd.collective_compute(
                        "AllGather", AluOp.bypass, replica_groups=ALL_GROUPS,
                        ins=[spl_loc[:]], outs=[spl_full[:]])

                # ---------------- main loop ----------------
                for blk2 in range(4):
                    for pre in (2 * blk2 + 2, 2 * blk2 + 3):
                        if pre < NBLK and pre not in g2tiles:
                            g2_gather(pre)
                    halves = [2 * blk2, 2 * blk2 + 1]
                    # g1 gather + pad
                    G1 = {}
                    for h, blk in enumerate(halves):
                        gb = sgw.tile([128, 512], F32, tag="g1b")
                        if do_compute:
                            V.memset(gb[:, 480:512], 0.0)
                        if do_gather:
                            col = blk * 120
                            for a in range(0, 15, GCH):
                                nk = min(GCH, 15 - a)
                                emit_dma_gather(
                                    nc.gpsimd,
                                    gb[:, 32 * a:32 * (a + nk)].rearrange("p (k c) -> p k c", c=32),
                                    spl_full[:, 0:32], g1i_sb[:, col:col + nk * 8],
                                    num_idxs=nk * 128, elem_size=32, elem_step=64,
                                    queue_num=next_q())
                                col += nk * 8
                        G1[h] = gb
                    if not do_compute:
                        for blk in halves:
                            del g2tiles[blk]
                        continue
                    # T1 transposes -> fp32r SBUF
                    T1 = []
                    for ch in range(4):
                        pt = ptr_p.tile([128, 256], F32, tag="ptr")
                        for h in range(2):
                            nc.tensor.transpose(out=pt[:, 128 * h:128 * (h + 1)],
                                                in_=G1[h][:, 128 * ch:128 * (ch + 1)],
                                                identity=ident[:])
                        t1c = srh.tile([128, 256], mm_dt, tag=f"t1c{ch}")
                        V.tensor_copy(out=t1c[:], in_=pt[:])
                        T1.append(t1c)
                    # A psum + sA (A + cb0) in fp32r
                    pA = pa_p.tile([32, 256], F32, tag="pa")
                    for ch in range(4):
                        nc.tensor.matmul(out=pA[:], lhsT=cw0A_r[:, 32 * ch:32 * ch + 32],
                                         rhs=T1[ch][:], start=(ch == 0), stop=(ch == 3))
                    sA = srh.tile([32, 256], mm_dt, tag="sA")
                    V.tensor_scalar(out=sA[:], in0=pA[:], scalar1=cb0_sb[:],
                                    scalar2=None, op0=AluOp.add)
                    py1 = py_p.tile([64, 256], F32, tag="py")
                    for f in range(FILT):
                        px = px_p.tile([32, 256], F32, tag="px")
                        for o in range(2):
                            J = 128 if o == 0 else 112
                            pt2 = ptr_p.tile([128, 256], F32, tag="ptr")
                            for h in range(2):
                                src = g2tiles[halves[h]][:, 15 * f + 8 * o:15 * f + 8 * o + (J // 16), :]
                                nc.tensor.transpose(
                                    out=pt2[0:J, 128 * h:128 * (h + 1)],
                                    in_=src.rearrange("p a b -> p (a b)"),
                                    identity=ident[:])
                            t2c = srh.tile([128, 256], mm_dt, tag="t2c")
                            nc.scalar.copy(out=t2c[0:J, :], in_=pt2[0:J, :])
                            nc.tensor.matmul(out=px[:], lhsT=cw0B_r[0:J, 32 * o:32 * o + 32],
                                             rhs=t2c[0:J, :], start=(o == 0), stop=False)
                        nc.tensor.matmul(out=px[:], lhsT=id32_r[:], rhs=sA[:],
                                         start=False, stop=True)
                        # x = leaky(px) ; px already has bias via sA
                        xm = swk.tile([32, 256], F32, tag="xm")
                        nc.scalar.mul(out=xm[:], in_=px[:], mul=NEG)
                        x_r = srh.tile([32, 256], mm_dt, tag="x_r")
                        V.tensor_tensor(out=x_r[:], in0=px[:], in1=xm[:], op=AluOp.max)
                        # x2 = leaky(cw1 @ x + cb1)
                        px2 = px2_p.tile([32, 256], F32, tag="px2")
                        nc.tensor.matmul(out=px2[:], lhsT=cw1_r[:], rhs=x_r[:],
                                         start=True, stop=True)
                        x2t = swk.tile([32, 256], F32, tag="x2t")
                        nc.scalar.activation(out=x2t[:], in_=px2[:],
                                             func=mybir.ActivationFunctionType.Identity,
                                             bias=cb1_sb[:], scale=1.0)
                        x2m = swk.tile([32, 256], F32, tag="x2m")
                        V.tensor_scalar(out=x2m[:], in0=x2t[:], scalar1=NEG,
                                        scalar2=None, op0=AluOp.mult)
                        x2_r = srh.tile([32, 256], mm_dt, tag="x2_r")
                        V.tensor_tensor(out=x2_r[:], in0=x2t[:], in1=x2m[:], op=AluOp.max)
                        nc.tensor.matmul(out=py1[:], lhsT=bw0_r[:, 64 * f:64 * f + 64],
                                         rhs=x2_r[:], start=(f == 0), stop=(f == FILT - 1))
                    # y1 = leaky(py1 + bb0) ; y = bw1 @ y1 + bb1
                    y1t = swk.tile([64, 256], F32, tag="y1t")
                    nc.scalar.activation(out=y1t[:], in_=py1[:],
                                         func=mybir.ActivationFunctionType.Identity,
                                         bias=bb0_sb[:], scale=1.0)
                    y1m = swk.tile([64, 256], F32, tag="y1m")
                    V.tensor_scalar(out=y1m[:], in0=y1t[:], scalar1=NEG,
                                    scalar2=None, op0=AluOp.mult)
                    y1_r = srh.tile([64, 256], mm_dt, tag="y1_r")
                    V.tensor_tensor(out=y1_r[:], in0=y1t[:], in1=y1m[:], op=AluOp.max)
                    py = py_p.tile([64, 256], F32, tag="py")
                    nc.tensor.matmul(out=py[:], lhsT=bw1_r[:], rhs=y1_r[:],
                                     start=True, stop=True)
                    y_sb = swk.tile([64, 256], F32, tag="y_sb")
                    nc.scalar.activation(out=y_sb[:], in_=py[:],
                                         func=mybir.ActivationFunctionType.Identity,
                                         bias=bb1_sb[:], scale=1.0)
                    nc.sync.dma_start(out=y_out[:, 256 * blk2:256 * (blk2 + 1)],
                                      in_=y_sb[:])
                    for blk in halves:
                        del g2tiles[blk]
    nc.compile()
    return nc


# ---------------------------------------------------------------- entry point
_CACHE = {}


def kernel(**inputs):
    """Full-input entry: shards across 8 NeuronCores, runs the Bass kernel,
    returns the full (2, 64, 4096) float32 output."""
    from concourse.bass_utils import run_bass_kernel_spmd
    shared, percore = host_prep(inputs)
    key = (shared["TTS"], tuple(shared["lsub"].tolist()))
    if key not in _CACHE:
        _CACHE[key] = build_nc(shared["TTS"], shared["lsub"],
                               shared["first"], shared["last"])
    nc = _CACHE[key]
    res = run_bass_kernel_spmd(nc, percore, core_ids=list(range(8)))
    out = np.zeros((B, 64, M1), np.float32)
    for c in range(8):
        out[c // SLICES, :, NSL * (c % SLICES):NSL * (c % SLICES + 1)] = \
            res.results[c]["y_out"]
    return out

